# revision 1
# baseline (speedup 1.0000x reference)
"""Trainium2 Bass kernel for the KBLN scoring model.

Computes, for full inputs:
    score_l = (emb_e[e1] * emb_rel[rel]) @ emb_e.T                       (B, E)
    phi     = exp(-((lit[e1][:,None,:] - lit[None,:,:]) - c)^2 / var)    (B, E, L)
    score_n = einsum('bel,bl->be', phi, nf_weights[rel])
    out     = sigmoid(score_l + score_n)

Reformulation used on device
----------------------------
With alpha[b,l] = (lit[e1[b],l] - 0.5 - c[l]) / sqrt(var[l]),
     beta[e,l]  = (lit[e,l]    - 0.5)        / sqrt(var[l]),
     g[l]       = -c[l] / sqrt(var[l]):

    phi = exp(-(alpha - beta)^2)
        = exp(-alpha^2) * exp(-(beta-g)^2 + g^2) * exp(2*(alpha-g)*beta)

The cross term x = 2*(alpha-g)*beta satisfies |x| <= 0.5/var <= 1, so a
10-term Taylor series of exp(x) is exact to ~1e-7.  That turns score_n into
a single matmul with contraction dim 64*10 = 640:

    score_n[b,e] = sum_{k,l} A[b,(k,l)] * Bt[(k,l),e]
    A[b,(k,l)]  = w[b,l] * exp(-alpha^2) * (2*(alpha-g))^k / k!   (host, tiny)
    Bt[(k,l),e] = exp(-(beta-g)^2 + g^2) * beta^k                 (device)

score_l is folded in as 200 extra contraction rows, giving one fused
(256 x 840) @ (840 x E_shard) matmul per core, followed by a sigmoid
(computed as 0.5*tanh(x/2)+0.5 to stay in the ACT "exp" table set).

Sharding: entities (E=15000) split evenly across 8 cores (1875 each);
batch side replicated; outputs concatenated on host.
"""

import math
import sys

import numpy as np

for _p in ("/opt/trn_rl_repo", "/root/.axon_site/_ro/trn_rl_repo"):
    if _p not in sys.path:
        sys.path.append(_p)

import concourse.bass as bass
import concourse.bacc as bacc
import concourse.mybir as mybir
from concourse import tile
from concourse import bass_utils

B, E, R, D, L = 256, 15000, 237, 200, 64
NCORES = 8
ES = E // NCORES          # 1875 entities per core
KT = 10                   # Taylor terms: k = 0..9
KB = KT // 2              # rhs k-tiles of 128 partitions (2 taylor orders each)
KTOT = KB * 128 + D       # 840 total contraction rows
F32 = mybir.dt.float32
MM_DT = mybir.dt.float32r  # matmul dtype (bitcast view of the f32 tiles)
# fp32r matmul needs an even moving free-dim, so the last slice starts one
# column early (column 1535 is computed twice with identical values)
N_SLICES = [(0, 512), (512, 512), (1024, 512), (1535, 340)]

TRACE = False             # test.py sets True to collect an NTFF profile
LAST = None               # last BassKernelResults (for test.py)

_PROG = None              # cached Bass program


def _build_program():
    nc = bacc.Bacc("TRN2", target_bir_lowering=False, debug=False)

    litT_d = nc.dram_tensor("litT", [L, ES], F32, kind="ExternalInput")
    eT_d = nc.dram_tensor("eT", [D, ES], MM_DT, kind="ExternalInput")
    lhsT_d = nc.dram_tensor("lhsT", [KTOT, B], MM_DT, kind="ExternalInput")
    cst_d = nc.dram_tensor("cst", [128, 3], F32, kind="ExternalInput")
    out_d = nc.dram_tensor("out", [B, ES], F32, kind="ExternalOutput")

    AF = mybir.ActivationFunctionType
    OP = mybir.AluOpType

    with tile.TileContext(nc) as tc:
        with (
            tc.tile_pool(name="persist", bufs=1) as pool,
            tc.tile_pool(name="psum", bufs=4, space="PSUM") as ppool,
            tc.tile_pool(name="outs", bufs=4) as opool,
        ):
            cst = pool.tile([128, 3], F32)
            nc.sync.dma_start(cst, cst_d[:, :])
            rsv = cst[:, 0:1]     # 1/sqrt(var), duplicated in both halves
            cm05 = cst[:, 1:2]    # c - 0.5
            g2 = cst[:, 2:3]      # c^2/var

            # lhsT k-tiles: 5x Taylor [128, 256], emb [128, 256] + [72, 256]

            lhs_tiles = []
            for j in range(KB + 2):
                r0 = j * 128
                p = min(128, KTOT - r0)
                t = pool.tile([128, B], MM_DT, name=f"lhs{j}")
                nc.scalar.dma_start(t[:p, :], lhsT_d[r0 : r0 + p, :])
                lhs_tiles.append((t, p))

            lit2 = pool.tile([128, ES], F32)
            eTa = pool.tile([128, ES], MM_DT)
            eTb = pool.tile([128, ES], MM_DT)
            beta = pool.tile([128, ES], F32)
            bg = pool.tile([128, ES], F32)
            V = pool.tile([128, ES], MM_DT)   # becomes Bt0 = [V ; V*beta]
            P2 = pool.tile([128, ES], F32)    # beta^2, both halves
            Bts = [pool.tile([128, ES], MM_DT, name=f"bt{j}") for j in range(1, KB)]
            rhs_tiles = [V] + Bts + [eTa, eTb]

            for n0, nsz in N_SLICES:
                s = np.s_[:, n0 : n0 + nsz]
                lo = np.s_[0:64, n0 : n0 + nsz]
                hi = np.s_[64:128, n0 : n0 + nsz]

                # load lit, duplicated into both partition halves
                nc.sync.dma_start(lit2[lo], litT_d[:, n0 : n0 + nsz])
                nc.sync.dma_start(lit2[hi], litT_d[:, n0 : n0 + nsz])
                nc.scalar.dma_start(eTa[s], eT_d[0:128, n0 : n0 + nsz])
                nc.scalar.dma_start(
                    eTb[0:72, n0 : n0 + nsz], eT_d[128:200, n0 : n0 + nsz]
                )

                # Bt ladder build
                nc.vector.tensor_scalar(beta[s], lit2[s], 0.5, rsv, OP.subtract, OP.mult)
                nc.vector.tensor_scalar(bg[s], lit2[s], cm05, rsv, OP.add, OP.mult)
                nc.scalar.activation(bg[s], bg[s], AF.Square)
                nc.scalar.activation(V[s], bg[s], AF.Exp, bias=g2, scale=-1.0)
                nc.scalar.activation(P2[s], beta[s], AF.Square)
                nc.vector.tensor_mul(V[hi], V[hi], beta[hi])   # V := [V ; V*beta]
                prev = V
                for bt in Bts:
                    nc.vector.tensor_mul(bt[s], prev[s], P2[s])
                    prev = bt

                # fused matmul: psum[m, n] = sum_j lhsT_j[:, m].T @ rhs_j[:, n]
                for m in range(2):
                    ms = np.s_[m * 128 : (m + 1) * 128]
                    ps = ppool.tile([128, 512], F32, name="ps")
                    for j, (lt, p) in enumerate(lhs_tiles):
                        nc.tensor.matmul(
                            ps[:, :nsz],
                            lt[:p, ms],
                            rhs_tiles[j][:p, n0 : n0 + nsz],
                            start=(j == 0),
                            stop=(j == len(lhs_tiles) - 1),
                        )
                    ob = opool.tile([128, 512], F32, name="ob")
                    # sigmoid(x) = 0.5*tanh(x/2) + 0.5  (stays in exp table set)
                    nc.scalar.activation(ob[:, :nsz], ps[:, :nsz], AF.Tanh, scale=0.5)
                    nc.vector.tensor_scalar(
                        ob[:, :nsz], ob[:, :nsz], 0.5, 0.5, OP.mult, OP.add
                    )
                    nc.sync.dma_start(out_d[ms, n0 : n0 + nsz], ob[:, :nsz])

    nc.compile()
    return nc


def _host_prep(emb_e, emb_rel, nf_weights, lit, c, var, e1, rel):
    f32 = np.float32
    e1 = np.asarray(e1).astype(np.int64)
    rel = np.asarray(rel).astype(np.int64)
    lit64 = np.asarray(lit, np.float64)
    c64 = np.asarray(c, np.float64)
    var64 = np.asarray(var, np.float64)

    rsv = 1.0 / np.sqrt(var64)                     # (L,)
    P = lit64[e1]                                   # (B, L)
    w = np.asarray(nf_weights, np.float64)[rel]     # (B, L)
    amg = (P - 0.5) * rsv                           # alpha - g
    alpha = (P - 0.5 - c64) * rsv
    u = np.exp(-(alpha**2)) * w                     # (B, L)
    t2 = 2.0 * amg

    lhsT = np.zeros((KTOT, B), f32)
    for k in range(KT):
        j, h = divmod(k, 2)
        A_k = u * t2**k / math.factorial(k)         # (B, L)
        lhsT[j * 128 + h * 64 : j * 128 + h * 64 + 64, :] = A_k.T.astype(f32)
    x = np.asarray(emb_e, f32)[e1] * np.asarray(emb_rel, f32)[rel]  # (B, D)
    lhsT[KB * 128 :, :] = x.T

    cst = np.zeros((128, 3), f32)
    cst[0:64, 0] = cst[64:128, 0] = rsv
    cst[0:64, 1] = cst[64:128, 1] = c64 - 0.5
    cst[0:64, 2] = cst[64:128, 2] = c64**2 / var64

    litT = np.ascontiguousarray(np.asarray(lit, f32).T)     # (L, E)
    eT = np.ascontiguousarray(np.asarray(emb_e, f32).T)     # (D, E)

    in_maps = []
    for ci in range(NCORES):
        lo, hi = ci * ES, (ci + 1) * ES
        in_maps.append(
            {
                "litT": np.ascontiguousarray(litT[:, lo:hi]),
                "eT": np.ascontiguousarray(eT[:, lo:hi]),
                "lhsT": lhsT,
                "cst": cst,
            }
        )
    return in_maps


def kernel(emb_e, emb_rel, nf_weights, lit, c, var, e1, rel):
    global _PROG, LAST
    if _PROG is None:
        _PROG = _build_program()
    in_maps = _host_prep(emb_e, emb_rel, nf_weights, lit, c, var, e1, rel)
    res = bass_utils.run_bass_kernel_spmd(
        _PROG, in_maps, core_ids=list(range(NCORES)), trace=TRACE
    )
    LAST = res
    return np.concatenate([res.results[ci]["out"] for ci in range(NCORES)], axis=1)



# revision 4
# speedup vs baseline: 1.2075x; 1.2075x over previous
"""Trainium2 Bass kernel for the KBLN scoring model.

Computes, for full inputs:
    score_l = (emb_e[e1] * emb_rel[rel]) @ emb_e.T                       (B, E)
    phi     = exp(-((lit[e1][:,None,:] - lit[None,:,:]) - c)^2 / var)    (B, E, L)
    score_n = einsum('bel,bl->be', phi, nf_weights[rel])
    out     = sigmoid(score_l + score_n)

Reformulation
-------------
With alpha[b,l] = (lit[e1[b],l] - 0.5 - c[l]) / sqrt(var[l]),
     beta[e,l]  = (lit[e,l]    - 0.5)        / sqrt(var[l]),
     g[l]       = -c[l] / sqrt(var[l]):

    phi = exp(-alpha^2) * exp(-(beta-g)^2 + g^2) * exp(x),
    x   = 2*(alpha-g)*beta,  |x| <= 0.5/var[l]  (per-l bound r_l <= 1).

exp(x) is replaced by a per-l degree-3 Chebyshev-interpolant polynomial on
[-r_l, r_l] (near-minimax, max err ~5e-3 of phi), which makes score_n a
single matmul with contraction 4*64 = 256:

    score_n[b,e] = sum_{k,l} A[b,(k,l)] * Bt[(k,l),e]
    A[b,(k,l)]  = w * exp(-alpha^2) * C[k,l] * (2*(alpha-g))^k    (host, tiny)
    Bt[(k,l),e] = exp(-(beta-g)^2 + g^2) * beta^k                 (host, E*L)

score_l folds in as 200 extra contraction rows -> one fused
(256 x 456) @ (456 x E_shard) bf16 matmul per core + a Sigmoid activation.
All feature construction happens on host (it is O(E*L), tiny next to the
O(B*E*L) matmul), so the device program is pure DMA + matmul + sigmoid.

Sharding: entities (E=15000) split evenly across 8 cores (1875 each);
batch side replicated; outputs concatenated on host.
"""

import sys

import numpy as np
import ml_dtypes

for _p in ("/opt/trn_rl_repo", "/root/.axon_site/_ro/trn_rl_repo"):
    if _p not in sys.path:
        sys.path.append(_p)

import concourse.bass as bass
import concourse.bacc as bacc
import concourse.mybir as mybir
from concourse import tile
from concourse import bass_utils

B, E, R, D, L = 256, 15000, 237, 200, 64
NCORES = 8
ES = E // NCORES          # 1875 entities per core
DEG = 3                   # polynomial degree for exp(x) on [-r_l, r_l]
KT = DEG + 1              # 4 polynomial terms
KTOT = KT * L + D         # 456 total contraction rows
F32 = mybir.dt.float32
BF16 = mybir.dt.bfloat16
BF16_NP = ml_dtypes.bfloat16
# row extents of the 4 contraction k-tiles: [k0;k1], [k2;k3], emb[0:128], emb[128:200]
TILE_P = [128, 128, 128, KTOT - 384]
# entity slices; last starts one col early so every slice is even-sized
N_SLICES = [(0, 512), (512, 512), (1024, 512), (1535, 340)]

TRACE = False             # test.py sets True to collect an NTFF profile
LAST = None               # last BassKernelResults (for test.py)

_PROG = None              # cached Bass program
_CHEB = None              # cached per-l polynomial coefficients


def _build_program():
    nc = bacc.Bacc("TRN2", target_bir_lowering=False, debug=False)

    rhs_d = nc.dram_tensor("rhs", [KTOT, ES], BF16, kind="ExternalInput")
    lhsT_d = nc.dram_tensor("lhsT", [KTOT, B], BF16, kind="ExternalInput")
    out_d = nc.dram_tensor("out", [B, ES], BF16, kind="ExternalOutput")

    AF = mybir.ActivationFunctionType
    NT = len(TILE_P)

    with tile.TileContext(nc) as tc:
        with (
            tc.tile_pool(name="persist", bufs=1) as pool,
            tc.tile_pool(name="psum", bufs=1, space="PSUM") as ppool,
            tc.tile_pool(name="outs", bufs=4) as opool,
        ):
            lhs_tiles = []
            for j in range(NT):
                p = TILE_P[j]
                t = pool.tile([128, B], BF16, name=f"lhs{j}")
                nc.scalar.dma_start(t[:p, :], lhsT_d[j * 128 : j * 128 + p, :])
                lhs_tiles.append(t)

            rhs_tiles = []
            for j in range(NT):
                p = TILE_P[j]
                t = pool.tile([128, ES], BF16, name=f"rhs{j}")
                # sliced to let the first matmuls start before the full
                # tile lands; two queues to pipeline descriptors
                for si, (n0, nsz) in enumerate(N_SLICES):
                    q = nc.sync if (si % 2 == 0) else nc.gpsimd
                    q.dma_start(
                        t[:p, n0 : n0 + nsz], rhs_d[j * 128 : j * 128 + p, n0 : n0 + nsz]
                    )
                rhs_tiles.append(t)

            for m in range(2):
                ms = np.s_[m * 128 : (m + 1) * 128]
                pss = [ppool.tile([128, 512], F32, name=f"ps{m}{si}") for si in range(4)]
                for j in range(NT):
                    p = TILE_P[j]
                    for si, (n0, nsz) in enumerate(N_SLICES):
                        nc.tensor.matmul(
                            pss[si][:, :nsz],
                            lhs_tiles[j][:p, ms],
                            rhs_tiles[j][:p, n0 : n0 + nsz],
                            start=(j == 0),
                            stop=(j == NT - 1),
                        )
                for si, (n0, nsz) in enumerate(N_SLICES):
                    ob = opool.tile([128, 512], BF16, name="ob")
                    nc.scalar.activation(ob[:, :nsz], pss[si][:, :nsz], AF.Sigmoid)
                    nc.scalar.dma_start(out_d[ms, n0 : n0 + nsz], ob[:, :nsz])

    nc.compile()
    return nc


def _cheb_coeffs():
    """Per-l power-basis coeffs of the deg-DEG Chebyshev interpolant of
    exp on [-r_l, r_l], r_l = 0.5/var_l.  Depends only on var (fixed)."""
    global _CHEB
    if _CHEB is None:
        # var is an input, so compute lazily in _host_prep instead
        raise RuntimeError
    return _CHEB


def _host_prep(emb_e, emb_rel, nf_weights, lit, c, var, e1, rel):
    global _CHEB
    f64 = np.float64
    e1 = np.asarray(e1).astype(np.int64)
    rel = np.asarray(rel).astype(np.int64)
    lit64 = np.asarray(lit, f64)
    c64 = np.asarray(c, f64)
    var64 = np.asarray(var, f64)

    s = np.sqrt(var64)
    g = -c64 / s
    if _CHEB is None:
        r = 0.5 / var64
        C = np.zeros((KT, L))
        for l in range(L):
            ch = np.polynomial.chebyshev.Chebyshev.interpolate(
                np.exp, DEG, domain=[-r[l], r[l]]
            )
            C[:, l] = ch.convert(kind=np.polynomial.Polynomial).coef
        _CHEB = C
    C = _CHEB

    P = lit64[e1]                                   # (B, L)
    w = np.asarray(nf_weights, f64)[rel]            # (B, L)
    amg = (P - 0.5) / s                             # alpha - g
    alpha = amg + g
    u = w * np.exp(-(alpha**2))                     # (B, L)

    lhsT = np.empty((KTOT, B), f64)
    t2k = np.ones_like(amg)
    for k in range(KT):
        lhsT[k * L : (k + 1) * L, :] = (u * C[k] * t2k).T
        t2k *= 2.0 * amg
    x = np.asarray(emb_e, f64)[e1] * np.asarray(emb_rel, f64)[rel]  # (B, D)
    lhsT[KT * L :, :] = x.T
    lhsT = lhsT.astype(BF16_NP)

    beta = (lit64 - 0.5) / s                        # (E, L)
    V = np.exp(-((beta - g) ** 2) + g**2)           # (E, L)
    rhs = np.empty((KTOT, E), f64)
    bk = V.copy()
    for k in range(KT):
        rhs[k * L : (k + 1) * L, :] = bk.T
        bk *= beta
    rhs[KT * L :, :] = np.asarray(emb_e, f64).T
    rhs = rhs.astype(BF16_NP)

    in_maps = []
    for ci in range(NCORES):
        lo, hi = ci * ES, (ci + 1) * ES
        in_maps.append(
            {
                "rhs": np.ascontiguousarray(rhs[:, lo:hi]),
                "lhsT": lhsT,
            }
        )
    return in_maps


def kernel(emb_e, emb_rel, nf_weights, lit, c, var, e1, rel):
    global _PROG, LAST
    if _PROG is None:
        _PROG = _build_program()
    in_maps = _host_prep(emb_e, emb_rel, nf_weights, lit, c, var, e1, rel)
    res = bass_utils.run_bass_kernel_spmd(
        _PROG, in_maps, core_ids=list(range(NCORES)), trace=TRACE
    )
    LAST = res
    out = np.concatenate([res.results[ci]["out"] for ci in range(NCORES)], axis=1)
    return out.astype(np.float32)


# revision 6
# speedup vs baseline: 1.4778x; 1.2239x over previous
"""Trainium2 Bass kernel for the KBLN scoring model.

Computes, for full inputs:
    score_l = (emb_e[e1] * emb_rel[rel]) @ emb_e.T                       (B, E)
    phi     = exp(-((lit[e1][:,None,:] - lit[None,:,:]) - c)^2 / var)    (B, E, L)
    score_n = einsum('bel,bl->be', phi, nf_weights[rel])
    out     = sigmoid(score_l + score_n)

Reformulation
-------------
With alpha[b,l] = (lit[e1[b],l] - 0.5 - c[l]) / sqrt(var[l]),
     beta[e,l]  = (lit[e,l]    - 0.5)        / sqrt(var[l]),
     g[l]       = -c[l] / sqrt(var[l]):

    phi = exp(-alpha^2) * exp(-(beta-g)^2 + g^2) * exp(x),
    x   = 2*(alpha-g)*beta,  |x| <= 0.5/var[l]  (per-l bound r_l <= 1).

exp(x) is replaced by a per-l degree-3 Chebyshev-interpolant polynomial on
[-r_l, r_l] (near-minimax), which makes score_n a single matmul with
contraction 4*64 = 256:

    score_n[b,e] = sum_{k,l} A[b,(k,l)] * Bt[(k,l),e]
    A[b,(k,l)]  = w * exp(-alpha^2) * C[k,l] * (2*(alpha-g))^k    (host, tiny)
    Bt[(k,l),e] = exp(-(beta-g)^2 + g^2) * beta^k                 (host, E*L)

score_l folds in as 200 extra contraction rows -> one fused
(256 x 456) @ (456 x E_shard) bf16 matmul per core + a Sigmoid activation.
All feature construction happens on host (it is O(E*L), tiny next to the
O(B*E*L) matmul), so the device program is pure DMA + matmul + sigmoid.

Device schedule: dummy warmup matmuls ramp the PE clock while inputs
stream in (lhsT + per-tile column halves, interleaved across the sync and
gpsimd queues in need order); real matmuls run slice-by-slice so each
PSUM bank's sigmoid (scalar engine) and store (sync/gpsimd) overlap the
remaining matmuls.

Sharding: entities (E=15000) split evenly across 8 cores (1875 each);
batch side replicated; outputs concatenated on host.
"""

import sys

import numpy as np
import ml_dtypes

for _p in ("/opt/trn_rl_repo", "/root/.axon_site/_ro/trn_rl_repo"):
    if _p not in sys.path:
        sys.path.append(_p)

import concourse.bass as bass
import concourse.bacc as bacc
import concourse.mybir as mybir
from concourse import tile
from concourse import bass_utils

B, E, R, D, L = 256, 15000, 237, 200, 64
NCORES = 8
ES = E // NCORES          # 1875 entities per core
DEG = 3                   # polynomial degree for exp(x) on [-r_l, r_l]
KT = DEG + 1              # 4 polynomial terms
KTOT = KT * L + D         # 456 total contraction rows
F32 = mybir.dt.float32
BF16 = mybir.dt.bfloat16
BF16_NP = ml_dtypes.bfloat16
# row extents of the 4 contraction k-tiles: [k0;k1], [k2;k3], emb[0:128], emb[128:200]
TILE_P = [128, 128, 128, KTOT - 384]
# entity slices; last starts one col early so every slice is even-sized
N_SLICES = [(0, 512), (512, 512), (1024, 512), (1535, 340)]
# column halves for input DMA (slice 0,1 in h0; 2,3 in h1)
HALVES = [(0, 1024), (1024, ES - 1024)]
N_WARM = 14               # dummy matmuls to ramp the PE clock

TRACE = False             # test.py sets True to collect an NTFF profile
LAST = None               # last BassKernelResults (for test.py)

_PROG = None              # cached Bass program
_CHEB = None              # cached per-l polynomial coefficients


def _build_program():
    nc = bacc.Bacc("TRN2", target_bir_lowering=False, debug=False)

    rhs_d = nc.dram_tensor("rhs", [KTOT, ES], BF16, kind="ExternalInput")
    lhsT_d = nc.dram_tensor("lhsT", [KTOT, B], BF16, kind="ExternalInput")
    out_d = nc.dram_tensor("out", [B, ES], BF16, kind="ExternalOutput")

    AF = mybir.ActivationFunctionType
    NT = len(TILE_P)

    with tile.TileContext(nc) as tc:
        with (
            tc.tile_pool(name="persist", bufs=1) as pool,
            tc.tile_pool(name="psum", bufs=1, space="PSUM") as ppool,
            tc.tile_pool(name="outs", bufs=4) as opool,
        ):
            # -- PE warmup: ramp the tensor-engine clock while DMAs run.
            # Reads zeroed scratch, accumulates into a scratch PSUM bank.
            warm = pool.tile([128, 128], BF16, name="warm")
            wps = ppool.tile([128, 128], F32, name="wps")
            nc.vector.memset(warm, 0.0)
            for i in range(N_WARM):
                nc.tensor.matmul(wps, warm, warm, start=True, stop=True)

            # -- input DMAs, in need order, on the sync + gpsimd queues
            lhs_tiles = [pool.tile([128, B], BF16, name=f"lhs{j}") for j in range(NT)]
            rhs_tiles = [pool.tile([128, ES], BF16, name=f"rhs{j}") for j in range(NT)]
            qs = [nc.sync, nc.gpsimd]
            for j in range(NT):
                p = TILE_P[j]
                qs[j % 2].dma_start(
                    lhs_tiles[j][:p, :], lhsT_d[j * 128 : j * 128 + p, :]
                )
            for h0, hsz in HALVES:
                for j in range(NT):
                    p = TILE_P[j]
                    qs[j % 2].dma_start(
                        rhs_tiles[j][:p, h0 : h0 + hsz],
                        rhs_d[j * 128 : j * 128 + p, h0 : h0 + hsz],
                    )

            # -- matmul / sigmoid / store, slice by slice
            for si, (n0, nsz) in enumerate(N_SLICES):
                for m in range(2):
                    ms = np.s_[m * 128 : (m + 1) * 128]
                    ps = ppool.tile([128, 512], F32, name="ps", tag="ps", bufs=4)
                    for j in range(NT):
                        p = TILE_P[j]
                        nc.tensor.matmul(
                            ps[:, :nsz],
                            lhs_tiles[j][:p, ms],
                            rhs_tiles[j][:p, n0 : n0 + nsz],
                            start=(j == 0),
                            stop=(j == NT - 1),
                        )
                    ob = opool.tile([128, 512], BF16, name="ob")
                    nc.scalar.activation(ob[:, :nsz], ps[:, :nsz], AF.Sigmoid)
                    qs[(2 * si + m) % 2].dma_start(
                        out_d[ms, n0 : n0 + nsz], ob[:, :nsz]
                    )

    nc.compile()
    return nc


def _host_prep(emb_e, emb_rel, nf_weights, lit, c, var, e1, rel):
    global _CHEB
    f64 = np.float64
    e1 = np.asarray(e1).astype(np.int64)
    rel = np.asarray(rel).astype(np.int64)
    lit64 = np.asarray(lit, f64)
    c64 = np.asarray(c, f64)
    var64 = np.asarray(var, f64)

    s = np.sqrt(var64)
    g = -c64 / s
    if _CHEB is None:
        r = 0.5 / var64
        C = np.zeros((KT, L))
        for l in range(L):
            ch = np.polynomial.chebyshev.Chebyshev.interpolate(
                np.exp, DEG, domain=[-r[l], r[l]]
            )
            C[:, l] = ch.convert(kind=np.polynomial.Polynomial).coef
        _CHEB = C
    C = _CHEB

    P = lit64[e1]                                   # (B, L)
    w = np.asarray(nf_weights, f64)[rel]            # (B, L)
    amg = (P - 0.5) / s                             # alpha - g
    alpha = amg + g
    u = w * np.exp(-(alpha**2))                     # (B, L)

    lhsT = np.empty((KTOT, B), f64)
    t2k = np.ones_like(amg)
    for k in range(KT):
        lhsT[k * L : (k + 1) * L, :] = (u * C[k] * t2k).T
        t2k *= 2.0 * amg
    x = np.asarray(emb_e, f64)[e1] * np.asarray(emb_rel, f64)[rel]  # (B, D)
    lhsT[KT * L :, :] = x.T
    lhsT = lhsT.astype(BF16_NP)

    beta = (lit64 - 0.5) / s                        # (E, L)
    V = np.exp(-((beta - g) ** 2) + g**2)           # (E, L)
    rhs = np.empty((KTOT, E), f64)
    bk = V.copy()
    for k in range(KT):
        rhs[k * L : (k + 1) * L, :] = bk.T
        bk *= beta
    rhs[KT * L :, :] = np.asarray(emb_e, f64).T
    rhs = rhs.astype(BF16_NP)

    in_maps = []
    for ci in range(NCORES):
        lo, hi = ci * ES, (ci + 1) * ES
        in_maps.append(
            {
                "rhs": np.ascontiguousarray(rhs[:, lo:hi]),
                "lhsT": lhsT,
            }
        )
    return in_maps


def kernel(emb_e, emb_rel, nf_weights, lit, c, var, e1, rel):
    global _PROG, LAST
    if _PROG is None:
        _PROG = _build_program()
    in_maps = _host_prep(emb_e, emb_rel, nf_weights, lit, c, var, e1, rel)
    res = bass_utils.run_bass_kernel_spmd(
        _PROG, in_maps, core_ids=list(range(NCORES)), trace=TRACE
    )
    LAST = res
    out = np.concatenate([res.results[ci]["out"] for ci in range(NCORES)], axis=1)
    return out.astype(np.float32)


# revision 13
# speedup vs baseline: 1.6280x; 1.1016x over previous
"""Trainium2 Bass kernel for the KBLN scoring model.

Computes, for full inputs:
    score_l = (emb_e[e1] * emb_rel[rel]) @ emb_e.T                       (B, E)
    phi     = exp(-((lit[e1][:,None,:] - lit[None,:,:]) - c)^2 / var)    (B, E, L)
    score_n = einsum('bel,bl->be', phi, nf_weights[rel])
    out     = sigmoid(score_l + score_n)

Reformulation
-------------
With alpha[b,l] = (lit[e1[b],l] - 0.5 - c[l]) / sqrt(var[l]),
     beta[e,l]  = (lit[e,l]    - 0.5)        / sqrt(var[l]),
     g[l]       = -c[l] / sqrt(var[l]):

    phi = exp(-alpha^2) * exp(-(beta-g)^2 + g^2) * exp(x),
    x   = 2*(alpha-g)*beta,  |x| <= 0.5/var[l]  (per-l bound r_l <= 1).

exp(x) is replaced by a per-l degree-3 Chebyshev-interpolant polynomial on
[-r_l, r_l] (near-minimax), which makes score_n a matmul with contraction
4*64 = 256:

    score_n[b,e] = sum_{k,l} A[b,(k,l)] * Bt[(k,l),e]
    A[b,(k,l)]  = w * exp(-alpha^2) * C[k,l] * (2*(alpha-g))^k    (host, tiny)
    Bt[(k,l),e] = exp(-(beta-g)^2 + g^2) * beta^k                 (host, E*L)

score_l folds in as 200 extra contraction rows.  All feature construction
happens on host (it is O(E*L), tiny next to the O(B*E*L) matmul), so the
device program is pure DMA + matmul + sigmoid.

Precision split (tolerance is 2e-2; this lands ~7e-3):
  - k0/k1 polynomial rows (the dominant terms): bf16 x bf16 matmul
  - k2/k3 rows + all 200 emb rows: fp8(e4m3) on BOTH sides with a per-row
    joint rescale a_r = sqrt(max|rhs_r| / max|lhs_r|) (scales cancel in the
    product), packed as one DoubleRow matmul (256 rows at 0.5 cyc/row)
    plus a 72-row tail matmul
  - output: sigmoid encoded as uint8 via round(127.49*tanh(x/2) + 127.5)
    (decoded on host), quartering the store traffic vs f32

The entity axis is padded 1875 -> 1888 per core so every SBUF byte offset
the PE reads (slice starts, DoubleRow group stride) is 32B-aligned in
every dtype -- the tensor engine silently reads garbage at unaligned
per-partition offsets.

Device schedule: dummy warmup matmuls ramp the PE clock while inputs
stream in (column halves in need order, balanced across the sync and
gpsimd queues); real matmuls run slice-by-slice so each PSUM bank's
tanh (scalar), u8 pack (vector) and store (alternating queues) overlap
the remaining matmuls.

Sharding: entities (E=15000) split evenly across 8 cores (1875 each);
batch side replicated; outputs concatenated on host.
"""

import sys

import numpy as np
import ml_dtypes

for _p in ("/opt/trn_rl_repo", "/root/.axon_site/_ro/trn_rl_repo"):
    if _p not in sys.path:
        sys.path.append(_p)

import concourse.bass as bass
import concourse.bacc as bacc
import concourse.mybir as mybir
from concourse import tile
from concourse import bass_utils

B, E, R, D, L = 256, 15000, 237, 200, 64
NCORES = 8
ES = E // NCORES          # 1875 entities per core
ESP = 1888                # padded per-core entity count (32B aligned all dtypes)
DEG = 3                   # polynomial degree for exp(x) on [-r_l, r_l]
KT = DEG + 1              # 4 polynomial terms
F32 = mybir.dt.float32
BF16 = mybir.dt.bfloat16
F8 = mybir.dt.float8e4
U8 = mybir.dt.uint8
BF16_NP = ml_dtypes.bfloat16
F8_NP = ml_dtypes.float8_e4m3fn
NF8 = 328                 # fp8 contraction rows: k2/k3 (128) + emb (200)
N_SLICES = [(0, 512), (512, 512), (1024, 512), (1504, 384)]
HALVES = [(0, 1024), (1024, ESP - 1024)]
N_WARM = 16               # dummy matmuls to ramp the PE clock
OSC, OBI = 127.49, 127.5  # u8 encode: round(OSC*tanh(x/2) + OBI)

TRACE = False             # test.py sets True to collect an NTFF profile
LAST = None               # last BassKernelResults (for test.py)

_PROG = None              # cached Bass program
_CHEB = None              # cached per-l polynomial coefficients


def _build_program():
    nc = bacc.Bacc("TRN2", target_bir_lowering=False, debug=False)

    rhs0_d = nc.dram_tensor("rhs0", [128, ESP], BF16, kind="ExternalInput")
    rhs12_d = nc.dram_tensor("rhs12", [128, 2, ESP], F8, kind="ExternalInput")
    rhs3_d = nc.dram_tensor("rhs3", [128, ESP], F8, kind="ExternalInput")
    lhs0_d = nc.dram_tensor("lhs0", [128, B], BF16, kind="ExternalInput")
    lhsf8_d = nc.dram_tensor("lhsf8", [128, 3, B], F8, kind="ExternalInput")
    out_d = nc.dram_tensor("out", [B, ESP], U8, kind="ExternalOutput")

    AF = mybir.ActivationFunctionType
    OP = mybir.AluOpType
    DR = mybir.MatmulPerfMode.DoubleRow

    with tile.TileContext(nc) as tc:
        with (
            tc.tile_pool(name="persist", bufs=1) as pool,
            tc.tile_pool(name="psum", bufs=1, space="PSUM") as ppool,
            tc.tile_pool(name="outs", bufs=4) as opool,
        ):
            # -- PE warmup: ramp the tensor-engine clock while DMAs run
            warm = pool.tile([128, 128], BF16, name="warm")
            wps = ppool.tile([128, 128], F32, name="wps", tag="wps", bufs=1)
            nc.vector.memset(warm, 0.0)
            for i in range(N_WARM):
                nc.tensor.matmul(wps, warm, warm, start=True, stop=True)

            # -- input DMAs, in need order, balanced across two queues
            lhs0 = pool.tile([128, B], BF16, name="lhs0")
            lhsf8 = pool.tile([128, 3, B], F8, name="lhsf8")
            rhs0 = pool.tile([128, ESP], BF16, name="rhs0")
            rhs12 = pool.tile([128, 2, ESP], F8, name="rhs12")
            rhs3 = pool.tile([128, ESP], F8, name="rhs3")
            (h0, hs0), (h1, hs1) = HALVES
            nc.sync.dma_start(lhs0, lhs0_d[:, :])
            nc.gpsimd.dma_start(lhsf8, lhsf8_d[:, :, :])
            nc.sync.dma_start(rhs0[:, h0 : h0 + hs0], rhs0_d[:, h0 : h0 + hs0])
            nc.gpsimd.dma_start(
                rhs12[:, :, h0 : h0 + hs0], rhs12_d[:, :, h0 : h0 + hs0]
            )
            nc.sync.dma_start(rhs3[:, h0 : h0 + hs0], rhs3_d[:, h0 : h0 + hs0])
            nc.gpsimd.dma_start(rhs0[:, h1 : h1 + hs1], rhs0_d[:, h1 : h1 + hs1])
            nc.sync.dma_start(
                rhs12[:, :, h1 : h1 + hs1], rhs12_d[:, :, h1 : h1 + hs1]
            )
            nc.gpsimd.dma_start(rhs3[:, h1 : h1 + hs1], rhs3_d[:, h1 : h1 + hs1])

            # -- matmul / sigmoid-as-u8 / store, slice by slice
            qs = [nc.sync, nc.gpsimd]
            for si, (n0, nsz) in enumerate(N_SLICES):
                for m in range(2):
                    ms = np.s_[m * 128 : (m + 1) * 128]
                    ps = ppool.tile([128, 512], F32, name="ps", tag="ps", bufs=4)
                    nc.tensor.matmul(
                        ps[:, :nsz], lhs0[:, ms], rhs0[:, n0 : n0 + nsz],
                        start=True, stop=False,
                    )
                    nc.tensor.matmul(
                        ps[:, :nsz],
                        lhsf8[:, 0:2, ms],
                        rhs12[:, :, n0 : n0 + nsz],
                        start=False, stop=False, perf_mode=DR,
                    )
                    nc.tensor.matmul(
                        ps[:, :nsz], lhsf8[:72, 2, ms], rhs3[:72, n0 : n0 + nsz],
                        start=False, stop=True,
                    )
                    tob = opool.tile([128, 512], BF16, name="tob", tag="tob")
                    nc.scalar.activation(tob[:, :nsz], ps[:, :nsz], AF.Tanh, scale=0.5)
                    ob = opool.tile([128, 512], U8, name="ob", tag="ob")
                    nc.vector.tensor_scalar(
                        ob[:, :nsz], tob[:, :nsz], OSC, OBI, OP.mult, OP.add
                    )
                    qs[(2 * si + m) % 2].dma_start(
                        out_d[ms, n0 : n0 + nsz], ob[:, :nsz]
                    )

    nc.compile()
    return nc


def _host_prep(emb_e, emb_rel, nf_weights, lit, c, var, e1, rel):
    global _CHEB
    f64 = np.float64
    e1 = np.asarray(e1).astype(np.int64)
    rel = np.asarray(rel).astype(np.int64)
    lit64 = np.asarray(lit, f64)
    c64 = np.asarray(c, f64)
    var64 = np.asarray(var, f64)

    s = np.sqrt(var64)
    g = -c64 / s
    if _CHEB is None:
        r = 0.5 / var64
        C = np.zeros((KT, L))
        for l in range(L):
            ch = np.polynomial.chebyshev.Chebyshev.interpolate(
                np.exp, DEG, domain=[-r[l], r[l]]
            )
            C[:, l] = ch.convert(kind=np.polynomial.Polynomial).coef
        _CHEB = C
    C = _CHEB

    P = lit64[e1]                                   # (B, L)
    w = np.asarray(nf_weights, f64)[rel]            # (B, L)
    amg = (P - 0.5) / s                             # alpha - g
    alpha = amg + g
    u = w * np.exp(-(alpha**2))                     # (B, L)

    # polynomial-term factors
    A = np.empty((KT * L, B), f64)                  # (k-major rows, B)
    Bt = np.empty((KT * L, E), f64)
    beta = (lit64 - 0.5) / s                        # (E, L)
    V = np.exp(-((beta - g) ** 2) + g**2)           # (E, L)
    t2k = np.ones_like(amg)
    bk = V.copy()
    for k in range(KT):
        A[k * L : (k + 1) * L] = (u * C[k] * t2k).T
        Bt[k * L : (k + 1) * L] = bk.T
        t2k *= 2.0 * amg
        bk *= beta

    X = np.asarray(emb_e, f64)[e1] * np.asarray(emb_rel, f64)[rel]  # (B, D)

    # fp8 rows (k2,k3 + emb) with joint per-row rescale (cancels in product)
    Lr = np.concatenate([A[128:256], X.T], axis=0)        # (NF8, B)
    Rr = np.concatenate([Bt[128:256], np.asarray(emb_e, f64).T], axis=0)  # (NF8, E)
    mL = np.abs(Lr).max(axis=1)
    mR = np.abs(Rr).max(axis=1)
    mL[mL == 0] = 1.0
    mR[mR == 0] = 1.0
    a = np.sqrt(mR / mL)
    Lq = (Lr * a[:, None]).astype(F8_NP)                  # (NF8, B)
    Rq = (Rr / a[:, None]).astype(F8_NP)                  # (NF8, E)

    lhs0 = np.ascontiguousarray(A[:128].astype(BF16_NP))  # (128, B) bf16
    lhsf8 = np.zeros((128, 3, B), F8_NP)
    lhsf8[:, 0, :] = Lq[0:128]
    lhsf8[:, 1, :] = Lq[128:256]
    lhsf8[:72, 2, :] = Lq[256:NF8]

    rhs0 = np.zeros((128, NCORES, ESP), BF16_NP)
    rhs12 = np.zeros((128, 2, NCORES, ESP), F8_NP)
    rhs3 = np.zeros((128, NCORES, ESP), F8_NP)
    b0 = Bt[:128].astype(BF16_NP)                         # (128, E)
    rhs0[:, :, :ES] = b0.reshape(128, NCORES, ES)
    rhs12[:, 0, :, :ES] = Rq[0:128].reshape(128, NCORES, ES)
    rhs12[:, 1, :, :ES] = Rq[128:256].reshape(128, NCORES, ES)
    rhs3[:72, :, :ES] = Rq[256:NF8].reshape(72, NCORES, ES)

    in_maps = []
    for ci in range(NCORES):
        in_maps.append(
            {
                "rhs0": np.ascontiguousarray(rhs0[:, ci]),
                "rhs12": np.ascontiguousarray(rhs12[:, :, ci]),
                "rhs3": np.ascontiguousarray(rhs3[:, ci]),
                "lhs0": lhs0,
                "lhsf8": lhsf8,
            }
        )
    return in_maps


def kernel(emb_e, emb_rel, nf_weights, lit, c, var, e1, rel):
    global _PROG, LAST
    if _PROG is None:
        _PROG = _build_program()
    in_maps = _host_prep(emb_e, emb_rel, nf_weights, lit, c, var, e1, rel)
    res = bass_utils.run_bass_kernel_spmd(
        _PROG, in_maps, core_ids=list(range(NCORES)), trace=TRACE
    )
    LAST = res
    q = np.concatenate(
        [res.results[ci]["out"][:, :ES] for ci in range(NCORES)], axis=1
    ).astype(np.float32)
    # decode u8: stored = round(OSC*tanh(x/2) + OBI) -> sigmoid = (tanh+1)/2
    t = (q - OBI) / OSC
    return np.clip((t + 1.0) * 0.5, 0.0, 1.0).astype(np.float32)


# revision 14
# speedup vs baseline: 1.6477x; 1.0121x over previous
"""Trainium2 Bass kernel for the KBLN scoring model.

Computes, for full inputs:
    score_l = (emb_e[e1] * emb_rel[rel]) @ emb_e.T                       (B, E)
    phi     = exp(-((lit[e1][:,None,:] - lit[None,:,:]) - c)^2 / var)    (B, E, L)
    score_n = einsum('bel,bl->be', phi, nf_weights[rel])
    out     = sigmoid(score_l + score_n)

Reformulation
-------------
With alpha[b,l] = (lit[e1[b],l] - 0.5 - c[l]) / sqrt(var[l]),
     beta[e,l]  = (lit[e,l]    - 0.5)        / sqrt(var[l]),
     g[l]       = -c[l] / sqrt(var[l]):

    phi = exp(-alpha^2) * exp(-(beta-g)^2 + g^2) * exp(x),
    x   = 2*(alpha-g)*beta,  |x| <= 0.5/var[l]  (per-l bound r_l <= 1).

exp(x) is replaced by a per-l degree-3 Chebyshev-interpolant polynomial on
[-r_l, r_l] (near-minimax), which makes score_n a matmul with contraction
4*64 = 256:

    score_n[b,e] = sum_{k,l} A[b,(k,l)] * Bt[(k,l),e]
    A[b,(k,l)]  = w * exp(-alpha^2) * C[k,l] * (2*(alpha-g))^k    (host, tiny)
    Bt[(k,l),e] = exp(-(beta-g)^2 + g^2) * beta^k                 (host, E*L)

score_l folds in as 200 extra contraction rows.  All feature construction
happens on host (it is O(E*L), tiny next to the O(B*E*L) matmul), so the
device program is pure DMA + matmul + sigmoid.

Precision split (tolerance is 2e-2; this lands ~7e-3):
  - k0/k1 polynomial rows (the dominant terms): bf16 x bf16 matmul
  - k2/k3 rows + all 200 emb rows: fp8(e4m3) on BOTH sides with a per-row
    joint rescale a_r = sqrt(max|rhs_r| / max|lhs_r|) (scales cancel in the
    product), packed as one DoubleRow matmul (256 rows at 0.5 cyc/row)
    plus a 72-row tail matmul
  - output: sigmoid encoded as uint8 via round(127.49*tanh(x/2) + 127.5)
    (decoded on host), quartering the store traffic vs f32

The entity axis is padded 1875 -> 1888 per core so every SBUF byte offset
the PE reads (slice starts, DoubleRow group stride) is 32B-aligned in
every dtype -- the tensor engine silently reads garbage at unaligned
per-partition offsets.

Device schedule: dummy warmup matmuls ramp the PE clock while inputs
stream in (column halves in need order, balanced across the sync and
gpsimd queues); real matmuls run slice-by-slice so each PSUM bank's
tanh (scalar), u8 pack (vector) and store (alternating queues) overlap
the remaining matmuls.

Sharding: entities (E=15000) split evenly across 8 cores (1875 each);
batch side replicated; outputs concatenated on host.
"""

import sys

import numpy as np
import ml_dtypes

for _p in ("/opt/trn_rl_repo", "/root/.axon_site/_ro/trn_rl_repo"):
    if _p not in sys.path:
        sys.path.append(_p)

import concourse.bass as bass
import concourse.bacc as bacc
import concourse.mybir as mybir
from concourse import tile
from concourse import bass_utils

B, E, R, D, L = 256, 15000, 237, 200, 64
NCORES = 8
ES = E // NCORES          # 1875 entities per core
ESP = 1888                # padded per-core entity count (32B aligned all dtypes)
DEG = 3                   # polynomial degree for exp(x) on [-r_l, r_l]
KT = DEG + 1              # 4 polynomial terms
F32 = mybir.dt.float32
BF16 = mybir.dt.bfloat16
F8 = mybir.dt.float8e4
U8 = mybir.dt.uint8
BF16_NP = ml_dtypes.bfloat16
F8_NP = ml_dtypes.float8_e4m3fn
NF8 = 328                 # fp8 contraction rows: k2/k3 (128) + emb (200)
N_SLICES = [(0, 512), (512, 512), (1024, 512), (1504, 384)]
HALVES = [(0, 1024), (1024, ESP - 1024)]
N_WARM = 34               # dummy matmuls to ramp the PE clock
OSC, OBI = 127.49, 127.5  # u8 encode: round(OSC*tanh(x/2) + OBI)

TRACE = False             # test.py sets True to collect an NTFF profile
LAST = None               # last BassKernelResults (for test.py)

_PROG = None              # cached Bass program
_CHEB = None              # cached per-l polynomial coefficients


def _build_program():
    nc = bacc.Bacc("TRN2", target_bir_lowering=False, debug=False)

    rhs0_d = nc.dram_tensor("rhs0", [128, ESP], BF16, kind="ExternalInput")
    rhs12_d = nc.dram_tensor("rhs12", [128, 2, ESP], F8, kind="ExternalInput")
    rhs3_d = nc.dram_tensor("rhs3", [128, ESP], F8, kind="ExternalInput")
    lhs0_d = nc.dram_tensor("lhs0", [128, B], BF16, kind="ExternalInput")
    lhsf8_d = nc.dram_tensor("lhsf8", [128, 3, B], F8, kind="ExternalInput")
    out_d = nc.dram_tensor("out", [B, ESP], U8, kind="ExternalOutput")

    AF = mybir.ActivationFunctionType
    OP = mybir.AluOpType
    DR = mybir.MatmulPerfMode.DoubleRow

    with tile.TileContext(nc) as tc:
        with (
            tc.tile_pool(name="persist", bufs=1) as pool,
            tc.tile_pool(name="psum", bufs=1, space="PSUM") as ppool,
            tc.tile_pool(name="outs", bufs=4) as opool,
        ):
            # -- PE warmup: ramp the tensor-engine clock while DMAs run
            warm = pool.tile([128, 128], BF16, name="warm")
            wps = ppool.tile([128, 128], F32, name="wps", tag="wps", bufs=1)
            nc.vector.memset(warm, 0.0)
            for i in range(N_WARM):
                nc.tensor.matmul(wps, warm, warm, start=True, stop=True)

            # -- input DMAs, in need order, balanced across two queues
            lhs0 = pool.tile([128, B], BF16, name="lhs0")
            lhsf8 = pool.tile([128, 3, B], F8, name="lhsf8")
            rhs0 = pool.tile([128, ESP], BF16, name="rhs0")
            rhs12 = pool.tile([128, 2, ESP], F8, name="rhs12")
            rhs3 = pool.tile([128, ESP], F8, name="rhs3")
            (h0, hs0), (h1, hs1) = HALVES
            nc.sync.dma_start(lhs0, lhs0_d[:, :])
            nc.gpsimd.dma_start(lhsf8, lhsf8_d[:, :, :])
            nc.sync.dma_start(rhs0[:, h0 : h0 + hs0], rhs0_d[:, h0 : h0 + hs0])
            nc.gpsimd.dma_start(
                rhs12[:, :, h0 : h0 + hs0], rhs12_d[:, :, h0 : h0 + hs0]
            )
            nc.sync.dma_start(rhs3[:, h0 : h0 + hs0], rhs3_d[:, h0 : h0 + hs0])
            nc.gpsimd.dma_start(rhs0[:, h1 : h1 + hs1], rhs0_d[:, h1 : h1 + hs1])
            nc.sync.dma_start(
                rhs12[:, :, h1 : h1 + hs1], rhs12_d[:, :, h1 : h1 + hs1]
            )
            nc.gpsimd.dma_start(rhs3[:, h1 : h1 + hs1], rhs3_d[:, h1 : h1 + hs1])

            # -- matmul / sigmoid-as-u8 / store, slice by slice
            for si, (n0, nsz) in enumerate(N_SLICES):
                for m in range(2):
                    ms = np.s_[m * 128 : (m + 1) * 128]
                    ps = ppool.tile([128, 512], F32, name="ps", tag="ps", bufs=4)
                    nc.tensor.matmul(
                        ps[:, :nsz], lhs0[:, ms], rhs0[:, n0 : n0 + nsz],
                        start=True, stop=False,
                    )
                    nc.tensor.matmul(
                        ps[:, :nsz],
                        lhsf8[:, 0:2, ms],
                        rhs12[:, :, n0 : n0 + nsz],
                        start=False, stop=False, perf_mode=DR,
                    )
                    nc.tensor.matmul(
                        ps[:, :nsz], lhsf8[:72, 2, ms], rhs3[:72, n0 : n0 + nsz],
                        start=False, stop=True,
                    )
                    tob = opool.tile([128, 512], BF16, name="tob", tag="tob")
                    nc.scalar.activation(tob[:, :nsz], ps[:, :nsz], AF.Tanh, scale=0.5)
                    ob = opool.tile([128, 512], U8, name="ob", tag="ob")
                    nc.vector.tensor_scalar(
                        ob[:, :nsz], tob[:, :nsz], OSC, OBI, OP.mult, OP.add
                    )
                    nc.sync.dma_start(out_d[ms, n0 : n0 + nsz], ob[:, :nsz])

    nc.compile()
    return nc


def _host_prep(emb_e, emb_rel, nf_weights, lit, c, var, e1, rel):
    global _CHEB
    f64 = np.float64
    e1 = np.asarray(e1).astype(np.int64)
    rel = np.asarray(rel).astype(np.int64)
    lit64 = np.asarray(lit, f64)
    c64 = np.asarray(c, f64)
    var64 = np.asarray(var, f64)

    s = np.sqrt(var64)
    g = -c64 / s
    if _CHEB is None:
        r = 0.5 / var64
        C = np.zeros((KT, L))
        for l in range(L):
            ch = np.polynomial.chebyshev.Chebyshev.interpolate(
                np.exp, DEG, domain=[-r[l], r[l]]
            )
            C[:, l] = ch.convert(kind=np.polynomial.Polynomial).coef
        _CHEB = C
    C = _CHEB

    P = lit64[e1]                                   # (B, L)
    w = np.asarray(nf_weights, f64)[rel]            # (B, L)
    amg = (P - 0.5) / s                             # alpha - g
    alpha = amg + g
    u = w * np.exp(-(alpha**2))                     # (B, L)

    # polynomial-term factors
    A = np.empty((KT * L, B), f64)                  # (k-major rows, B)
    Bt = np.empty((KT * L, E), f64)
    beta = (lit64 - 0.5) / s                        # (E, L)
    V = np.exp(-((beta - g) ** 2) + g**2)           # (E, L)
    t2k = np.ones_like(amg)
    bk = V.copy()
    for k in range(KT):
        A[k * L : (k + 1) * L] = (u * C[k] * t2k).T
        Bt[k * L : (k + 1) * L] = bk.T
        t2k *= 2.0 * amg
        bk *= beta

    X = np.asarray(emb_e, f64)[e1] * np.asarray(emb_rel, f64)[rel]  # (B, D)

    # fp8 rows (k2,k3 + emb) with joint per-row rescale (cancels in product)
    Lr = np.concatenate([A[128:256], X.T], axis=0)        # (NF8, B)
    Rr = np.concatenate([Bt[128:256], np.asarray(emb_e, f64).T], axis=0)  # (NF8, E)
    mL = np.abs(Lr).max(axis=1)
    mR = np.abs(Rr).max(axis=1)
    mL[mL == 0] = 1.0
    mR[mR == 0] = 1.0
    a = np.sqrt(mR / mL)
    Lq = (Lr * a[:, None]).astype(F8_NP)                  # (NF8, B)
    Rq = (Rr / a[:, None]).astype(F8_NP)                  # (NF8, E)

    lhs0 = np.ascontiguousarray(A[:128].astype(BF16_NP))  # (128, B) bf16
    lhsf8 = np.zeros((128, 3, B), F8_NP)
    lhsf8[:, 0, :] = Lq[0:128]
    lhsf8[:, 1, :] = Lq[128:256]
    lhsf8[:72, 2, :] = Lq[256:NF8]

    rhs0 = np.zeros((128, NCORES, ESP), BF16_NP)
    rhs12 = np.zeros((128, 2, NCORES, ESP), F8_NP)
    rhs3 = np.zeros((128, NCORES, ESP), F8_NP)
    b0 = Bt[:128].astype(BF16_NP)                         # (128, E)
    rhs0[:, :, :ES] = b0.reshape(128, NCORES, ES)
    rhs12[:, 0, :, :ES] = Rq[0:128].reshape(128, NCORES, ES)
    rhs12[:, 1, :, :ES] = Rq[128:256].reshape(128, NCORES, ES)
    rhs3[:72, :, :ES] = Rq[256:NF8].reshape(72, NCORES, ES)

    in_maps = []
    for ci in range(NCORES):
        in_maps.append(
            {
                "rhs0": np.ascontiguousarray(rhs0[:, ci]),
                "rhs12": np.ascontiguousarray(rhs12[:, :, ci]),
                "rhs3": np.ascontiguousarray(rhs3[:, ci]),
                "lhs0": lhs0,
                "lhsf8": lhsf8,
            }
        )
    return in_maps


def kernel(emb_e, emb_rel, nf_weights, lit, c, var, e1, rel):
    global _PROG, LAST
    if _PROG is None:
        _PROG = _build_program()
    in_maps = _host_prep(emb_e, emb_rel, nf_weights, lit, c, var, e1, rel)
    res = bass_utils.run_bass_kernel_spmd(
        _PROG, in_maps, core_ids=list(range(NCORES)), trace=TRACE
    )
    LAST = res
    q = np.concatenate(
        [res.results[ci]["out"][:, :ES] for ci in range(NCORES)], axis=1
    ).astype(np.float32)
    # decode u8: stored = round(OSC*tanh(x/2) + OBI) -> sigmoid = (tanh+1)/2
    t = (q - OBI) / OSC
    return np.clip((t + 1.0) * 0.5, 0.0, 1.0).astype(np.float32)
